# revision 32
# baseline (speedup 1.0000x reference)
# Bass/Tile kernel for nn_LongTermAttention (continuous long-term attention
# with rectangular basis functions) on 8 Trainium2 NeuronCores.
#
# Mathematical rewrite (verified exact vs the reference):
#   * G = F^T (F F^T + ridge I)^{-1} for the rectangular basis on the padded
#     uniform grid collapses to G[l, n] = (1/4.5) * [l // 4 == n], so
#     Bc[b,n,e] = (1/4.5) * sum_{j<4} k[b,e,4n+j]  (4-wide sum pooling).
#   * psi on the integration grid is a one-hot selector, so the P=1000-point
#     continuous softmax reduces to basis space with quadrature mass Wn per
#     basis:  p_n = exp(s_n) Wn / Z,  Z = sum_n exp(s_n) Wn + w_last.
#     Wn is folded into the VALUES tiles (and the Z "ones" column), so the
#     device computes a bias-free exp.
#   * The max-subtraction in the reference cancels exactly.
#
# Performance structure (v3):
#   * k stays in original [b, e, l] layout; pooling via single-pass
#     vector.pool_avg (innermost-dim reduce, x4 folded into weight scales),
#     with gpsimd covering half-tiles via a 2-op add tree.
#   * All SBUF tiles are persistent (unique tags) so no DMA trigger ever
#     blocks on tile recycling.
#   * Scores for head-pair m are emitted right after keysT[m] drains, so the
#     scalar-engine exp stream (the critical tail) starts as early as
#     possible; exp is the ONLY scalar work.
#   * PE warm-up matmuls (zeros) ramp the tensor-engine p-state before real
#     data lands; emission order keeps the PE busy throughout.
#   * Normalize: vector does psum->sbuf cast + Z reciprocal; gpsimd does the
#     per-head scaling from SBUF.
#
# Sharding: data-parallel over batch, 2 batches per core; weights replicated.

import numpy as np

B_FULL = 16
N_CORES = 8
B_PER = B_FULL // N_CORES  # 2
E = 512          # embed dim
L = 2048         # memory length
T = 256          # query length
N = 512          # basis count
H = 8            # heads
D = 64           # head dim
P_GRID = 1000    # integration points
RIDGE_C = 4.5    # F F^T diag (4.0) + ridge (0.5)

N_WARM1 = 12     # junk matmuls before first real matmul
N_WARM2 = 3      # junk matmuls inside the first keysT chain gap
N_WARM3 = 6      # junk matmuls while waiting for late b0 k tiles

_CACHE = {}


def _host_constants(Wk, Wv):
    """Pre-scale and lay out weights; build quadrature-mass vectors.
    Scales fold: pooling 1/4.5 and query scale 1/8 into Wk; 1/4.5 into Wv
    (device pooling is a plain 4-wide SUM)."""
    import ml_dtypes
    bf16 = ml_dtypes.bfloat16
    wk = (Wk.astype(np.float64) / (RIDGE_C * 8.0)).astype(np.float32)
    wv = (Wv.astype(np.float64) / RIDGE_C).astype(np.float32)
    # layout [p, kk, e'] with e = kk*128 + p
    wk_l = np.ascontiguousarray(
        wk.reshape(4, 128, 512).transpose(1, 0, 2)).astype(bf16)
    wv_l = np.ascontiguousarray(
        wv.reshape(4, 128, 512).transpose(1, 0, 2)).astype(bf16)
    # quadrature mass per basis (trapezoid weights summed per bin, p<999)
    p = np.arange(P_GRID)
    nmap = (512 * p) // 999
    w = np.full(P_GRID, 1.0 / 999.0)
    w[0] = w[-1] = 1.0 / 1998.0
    Wn = np.zeros(N)
    for i in range(P_GRID - 1):
        Wn[nmap[i]] += w[i]
    # wn8[p, ms, h] = Wn[ms*128 + p] replicated over 8 heads
    wn8 = np.ascontiguousarray(
        np.repeat(Wn.reshape(4, 128).T[:, :, None], H, axis=2)
    ).astype(np.float32)
    w_last = float(w[-1])
    return wk_l, wv_l, wn8, w_last


def _build_program(w_last):
    import concourse.bass as bass
    import concourse.mybir as mybir
    import concourse.tile as tile
    from concourse import bacc

    nc = bacc.Bacc(
        "TRN2",
        target_bir_lowering=False,
        debug=False,
        enable_asserts=False,
        num_devices=N_CORES,
    )

    f32 = mybir.dt.float32
    bf16 = mybir.dt.bfloat16

    k_d = nc.dram_tensor("k", [B_PER, 4, 128, L], bf16,
                         kind="ExternalInput").ap()
    q_d = nc.dram_tensor("q", [B_PER, 128, 4, T], bf16,
                         kind="ExternalInput").ap()
    wk_d = nc.dram_tensor("wk", [128, 4, E], bf16, kind="ExternalInput").ap()
    wv_d = nc.dram_tensor("wv", [128, 4, E], bf16, kind="ExternalInput").ap()
    wn8_d = nc.dram_tensor("wn8", [128, 4, H], f32, kind="ExternalInput").ap()
    out_d = nc.dram_tensor("out", [B_PER, T, E], f32,
                           kind="ExternalOutput").ap()

    from contextlib import ExitStack
    with tile.TileContext(nc) as tc, ExitStack() as ctx:
        _kernel_body(ctx, tc, nc, mybir, k_d, q_d, wk_d, wv_d, wn8_d, out_d,
                     w_last)

    nc.compile()
    return nc


def _kernel_body(ctx, tc, nc, mybir, k_d, q_d, wk_d, wv_d, wn8_d, out_d,
                 w_last):
    f32 = mybir.dt.float32
    bf16 = mybir.dt.bfloat16
    Exp = mybir.ActivationFunctionType.Exp
    AVG = mybir.PoolFunctionType.avg

    def pool(name, bufs, space="SBUF"):
        return ctx.enter_context(tc.tile_pool(name=name, bufs=bufs,
                                              space=space))

    # every SBUF tile is persistent (unique tag) so nothing ever blocks on
    # tile recycling
    sb = pool("sb", 1)
    t1p = pool("t1p", 4)
    ps_proj = pool("ps_proj", 2, "PSUM")  # [128, 512] f32 (1 bank each)
    ps_sc = pool("ps_sc", 2, "PSUM")      # [128, 1024] f32 (2 banks each)
    ps_ctx = pool("ps_ctx", 2, "PSUM")    # [128, 260] f32

    def sbt(shape, dtype, tag):
        return sb.tile(shape, dtype, tag=tag, name=tag)

    # ---------------- constants / DMA priority queues ----------------
    # act ring:  wn8, wk_h0, k(b0,1)h0, wk_h1, k(b0,1)h1, wv, k(b1,3), b0-out
    # sync ring: k(b0,0)h0, k(b0,2)h0, k(b0,0)h1, k(b0,2)h1, k(b1,0),
    #            k(b1,2), q1, b1-out
    # gpsimd SW ring: q0, k(b0,3), k(b1,1)
    wn8_sb = sbt([128, 4 * H], f32, "wn8")
    wk_sb = sbt([128, 4 * E], bf16, "wk")
    nc.scalar.dma_start(wk_sb[:].rearrange("p (kk e) -> p kk e", kk=4),
                        wk_d[:])
    wv_sb = sbt([128, 4 * E], bf16, "wv")
    qT = [sbt([128, 4 * T], bf16, f"qT{b}") for b in range(B_PER)]
    junk_sb = sbt([128, 512], bf16, "junk")
    nc.vector.memset(junk_sb[:], 0.0)

    # k tiles: kth[(b, et, h)] is the SBUF view of half h (n-range
    # h*256:(h+1)*256, i.e. l-range h*1024:(h+1)*1024)
    kth = {}

    def dma_k_full(b, et, eng):
        kt = sbt([128, L], bf16, f"k{b}_{et}")
        eng.dma_start(kt[:], k_d[b, et])
        kth[(b, et, 0)] = kt[:, 0:1024]
        kth[(b, et, 1)] = kt[:, 1024:2048]

    # sync handles two tiles fast; act takes wk+et1; the slow SW ring
    # gets et3 first (lands ~15us) plus b1's most-slack tile
    dma_k_full(0, 3, nc.gpsimd)
    dma_k_full(1, 1, nc.gpsimd)
    # sync ring
    dma_k_full(0, 0, nc.sync)
    dma_k_full(0, 2, nc.sync)
    nc.sync.dma_start(qT[0][:].rearrange("p (e t) -> p e t", e=4), q_d[0])
    nc.sync.dma_start(wv_sb[:].rearrange("p (kk e) -> p kk e", kk=4),
                      wv_d[:])
    dma_k_full(1, 0, nc.sync)
    dma_k_full(1, 2, nc.sync)
    nc.sync.dma_start(qT[1][:].rearrange("p (e t) -> p e t", e=4), q_d[1])
    # act ring (wk above; wn8 is only needed by the Z columns ~20us in)
    dma_k_full(0, 1, nc.scalar)
    nc.scalar.dma_start(wn8_sb[:].rearrange("p (m h) -> p m h", m=4),
                        wn8_d[:])
    dma_k_full(1, 3, nc.scalar)

    # ---------------- values tiles + persistent Z columns ----------------
    values = {(b, ms): sbt([128, H * 65], bf16, f"val{b}_{ms}")
              for b in range(B_PER) for ms in range(4)}
    wn8v = wn8_sb[:].rearrange("p (m h) -> p m h", m=4)

    def z_cols():
        for b in range(B_PER):
            for ms in range(4):
                vv = values[(b, ms)][:].rearrange("p (h c) -> p h c", c=65)
                nc.gpsimd.tensor_copy(vv[:, :, 64], wn8v[:, ms, :])

    # ---------------- pooling (4-wide sum; per n-half) ----------------
    # pl[(b, et, c)] [128, 256]; vector handles all c=0 halves via
    # tensor_reduce, gpsimd all c=1 halves via a 2-op add tree.
    pl = {(b, et, c): sbt([128, 256], bf16, f"pl{b}_{et}_{c}")
          for b in range(B_PER) for et in range(4) for c in range(2)}
    AXX = mybir.AxisListType.X
    ADD = mybir.AluOpType.add

    def pool_v(b, et, c):
        kv = kth[(b, et, c)].rearrange("p (n j) -> p n j", j=4)
        with nc.allow_low_precision("4-wide pooling sum in bf16"):
            nc.vector.tensor_reduce(pl[(b, et, c)][:], kv, axis=AXX, op=ADD)

    def pool_g(b, et, c):
        kv = kth[(b, et, c)].rearrange("p (n j) -> p n j", j=4)
        t1 = t1p.tile([128, 512], bf16, tag="t1", name=f"t1_{b}_{et}_{c}")
        t1v = t1[:].rearrange("p (n j) -> p n j", j=2)
        nc.gpsimd.tensor_add(t1v[:, :, :], kv[:, :, 0:2], kv[:, :, 2:4])
        nc.gpsimd.tensor_add(pl[(b, et, c)][:], t1v[:, :, 0], t1v[:, :, 1])

    # b0 pooling: c0 halves split across vector (et0,et2) and gpsimd
    # (et1,et3) so the critical first half completes fastest; c1 halves of
    # et1/et3 also on gpsimd (emitted now), et0/et2-c1 on vector later so
    # the keysT-c0 drains can jump the vector queue.
    pool_v(0, 0, 0)
    pool_v(0, 2, 0)
    pool_v(0, 3, 0)
    pool_g(0, 1, 0)
    pool_g(0, 1, 1)
    pool_g(0, 3, 1)
    z_cols()

    # ---------------- PE warm-up ----------------
    def junk_block(n):
        ps = ps_proj.tile([128, 512], f32, tag="ps_proj", name="junk")
        for _ in range(n):
            nc.tensor.matmul(ps[:], junk_sb[:, 0:128], junk_sb[:],
                             start=True, stop=True, skip_group_check=True)

    # ---------------- projections (per half) ----------------
    keysT = {}

    def kT_chain(b, m, c, order, warm=0):
        ps = ps_proj.tile([128, 512], f32, tag="ps_proj",
                          name=f"kT{b}_{m}_{c}")
        for i, kk in enumerate(order):
            if warm and i == 2:
                junk_block(warm)
            nc.tensor.matmul(
                ps[:, 0:256],
                wk_sb[:, kk * E + m * 128: kk * E + (m + 1) * 128],
                pl[(b, kk, c)][:],
                start=(i == 0), stop=(i == 3),
            )
        kt_sb = sbt([128, 256], bf16, f"kT{b}_{m}_{c}")
        nc.vector.tensor_copy(kt_sb[:], ps[:, 0:256])
        keysT[(b, m, c)] = kt_sb

    def val_chain(b, ms, order):
        ps = ps_proj.tile([128, 512], f32, tag="ps_proj", name=f"v{b}_{ms}")
        for i, kk in enumerate(order):
            nc.tensor.matmul(
                ps[:],
                pl[(b, kk, ms // 2)][:, (ms % 2) * 128:(ms % 2) * 128 + 128],
                wv_sb[:, kk * E:(kk + 1) * E],
                start=(i == 0), stop=(i == 3),
            )
        vv = values[(b, ms)][:].rearrange("p (h c) -> p h c", c=65)
        nc.vector.tensor_scalar_mul(
            vv[:, :, 0:64],
            ps[:].rearrange("p (h d) -> p h d", d=64),
            wn8_sb[:, ms * H: ms * H + 1],
        )

    # ---------------- scores + exp ----------------
    u_tiles = {}

    def score_block(b, hp, ab):
        ps = ps_sc.tile([128, 1024], f32, tag="ps_sc", name=f"sc{b}_{hp}_{ab}")
        for nbp in range(2):
            for h01 in range(2):
                nc.tensor.matmul(
                    ps[:, h01 * 512 + nbp * 256: h01 * 512 + nbp * 256 + 256],
                    keysT[(b, hp, ab)][h01 * 64:(h01 + 1) * 64,
                                       nbp * 128:(nbp + 1) * 128],
                    qT[b][h01 * 64:(h01 + 1) * 64,
                          hp * 256:(hp + 1) * 256],
                    start=True, stop=True,
                    skip_group_check=True,
                )
        u = sbt([128, 1024], bf16, f"u{b}_{hp}_{ab}")
        nc.scalar.activation(u[:], ps[:], Exp)
        u_tiles[(b, hp, ab)] = u

    # ---------------- ctx + normalize ----------------
    out_sbs = {(b, mb): sbt([128, E], f32, f"out{b}_{mb}")
               for b in range(B_PER) for mb in range(2)}
    Copy = mybir.ActivationFunctionType.Copy

    def ctx_block(b, hp):
        pc = ps_ctx.tile([128, 260], f32, tag="ps_ctx", name=f"ctx{b}_{hp}")
        for h01 in range(2):
            h = hp * 2 + h01
            for mb in range(2):
                c = h01 * 2 + mb
                for nb in range(4):
                    nc.tensor.matmul(
                        pc[:, c * 65:(c + 1) * 65],
                        u_tiles[(b, hp, nb // 2)][
                            :, h01 * 512 + (nb % 2) * 256 + mb * 128:
                            h01 * 512 + (nb % 2) * 256 + (mb + 1) * 128],
                        values[(b, nb)][:, h * 65: h * 65 + 65],
                        start=(nb == 0), stop=(nb == 3),
                        skip_group_check=True,
                    )
        pcv = pc[:].rearrange("p (c x) -> p c x", x=65)
        rz = sbt([128, 4], f32, f"rz{b}_{hp}")
        nc.vector.tensor_scalar_add(rz[:], pcv[:, :, 64], w_last)
        rzi = sbt([128, 4], f32, f"rzi{b}_{hp}")
        nc.vector.reciprocal(rzi[:], rz[:])
        for h01 in range(2):
            h = hp * 2 + h01
            for mb in range(2):
                c = h01 * 2 + mb
                dst = out_sbs[(b, mb)][:, h * 64:(h + 1) * 64]
                nc.vector.tensor_scalar_mul(dst, pcv[:, c, 0:64],
                                            rzi[:, c:c + 1])

    # ---------------- emission ----------------
    C0_B0 = (0, 2, 3, 1)    # b0 c0-half pooled-arrival order
    C1_B0 = (0, 2, 1, 3)    # b0 c1-half pooled-arrival order
    C0_B1 = (1, 3, 0, 2)
    C1_B1 = (0, 2, 1, 3)

    junk_block(N_WARM1)
    # b0 keysT c0 chains; m0/m1 drains jump the vector queue before the
    # vector picks up the et0/et2 c1-half reductions
    kT_chain(0, 0, 0, C0_B0, warm=N_WARM2)
    kT_chain(0, 1, 0, C0_B0)
    pool_v(0, 0, 1)
    pool_v(0, 2, 1)
    kT_chain(0, 2, 0, C0_B0)
    kT_chain(0, 3, 0, C0_B0)
    # all four A score blocks are ready now; emit them ahead of the
    # (possibly stalling) c1 keysT chains so the exp stream starts early
    score_block(0, 0, 0)
    score_block(0, 1, 0)
    kT_chain(0, 0, 1, C1_B0)
    score_block(0, 2, 0)
    kT_chain(0, 1, 1, C1_B0)
    score_block(0, 3, 0)
    kT_chain(0, 2, 1, C1_B0)
    kT_chain(0, 3, 1, C1_B0)
    # b1 pooling: gpsimd trees for et1/et3 c0 and all c1; vector c0 reds
    # for et0/et2 come after the b0 value drains (emitted below)
    pool_g(1, 1, 0)
    pool_g(1, 3, 0)
    pool_g(1, 1, 1)
    pool_g(1, 3, 1)
    pool_g(1, 0, 1)
    pool_g(1, 2, 1)
    pool_v(1, 0, 0)
    pool_v(1, 2, 0)
    # b1 keysT-c0 chains fill between the b0 B-score blocks; the b0 value
    # chains (needed only by ctx) fill between the b1 A-score blocks
    score_block(0, 0, 1)
    kT_chain(1, 0, 0, C0_B1)
    score_block(0, 1, 1)
    kT_chain(1, 1, 0, C0_B1)
    score_block(0, 2, 1)
    kT_chain(1, 2, 0, C0_B1)
    score_block(0, 3, 1)
    kT_chain(1, 3, 0, C0_B1)
    score_block(1, 0, 0)
    val_chain(0, 0, C0_B0)
    score_block(1, 1, 0)
    val_chain(0, 1, C0_B0)
    score_block(1, 2, 0)
    val_chain(0, 2, C1_B0)
    score_block(1, 3, 0)
    val_chain(0, 3, C1_B0)
    kT_chain(1, 0, 1, C1_B1)
    kT_chain(1, 1, 1, C1_B1)
    ctx_block(0, 0)
    kT_chain(1, 2, 1, C1_B1)
    kT_chain(1, 3, 1, C1_B1)
    ctx_block(0, 1)
    score_block(1, 0, 1)
    val_chain(1, 0, C0_B1)
    score_block(1, 1, 1)
    val_chain(1, 1, C0_B1)
    for mb in range(2):
        nc.scalar.dma_start(out_d[0, mb * 128:(mb + 1) * 128, 0:256],
                            out_sbs[(0, mb)][:, 0:256])
    ctx_block(0, 2)
    score_block(1, 2, 1)
    val_chain(1, 2, C1_B1)
    score_block(1, 3, 1)
    val_chain(1, 3, C1_B1)
    ctx_block(0, 3)
    for mb in range(2):
        nc.scalar.dma_start(out_d[0, mb * 128:(mb + 1) * 128, 256:512],
                            out_sbs[(0, mb)][:, 256:512])

    ctx_block(1, 0)
    ctx_block(1, 1)
    for mb in range(2):
        nc.sync.dma_start(out_d[1, mb * 128:(mb + 1) * 128, 0:256],
                          out_sbs[(1, mb)][:, 0:256])
    ctx_block(1, 2)
    ctx_block(1, 3)
    for mb in range(2):
        nc.sync.dma_start(out_d[1, mb * 128:(mb + 1) * 128, 256:512],
                          out_sbs[(1, mb)][:, 256:512])


def _get_program(w_last):
    if "nc" not in _CACHE:
        _CACHE["nc"] = _build_program(w_last)
    return _CACHE["nc"]


def make_in_maps(k, q, Wk, Wv):
    import ml_dtypes
    bf16 = ml_dtypes.bfloat16
    wk_l, wv_l, wn8, w_last = _host_constants(Wk, Wv)
    k16 = np.asarray(k).astype(bf16)
    q16 = np.asarray(q).astype(bf16)
    in_maps = []
    for c in range(N_CORES):
        ks = np.ascontiguousarray(
            k16[c * B_PER:(c + 1) * B_PER].reshape(B_PER, 4, 128, L))
        qs = q16[c * B_PER:(c + 1) * B_PER]          # [2, 256, 512]
        # -> [b, p, eb, t]
        qp = np.ascontiguousarray(
            qs.transpose(0, 2, 1).reshape(B_PER, 4, 128, T)
            .transpose(0, 2, 1, 3))
        in_maps.append({
            "k": ks,
            "q": qp,
            "wk": wk_l,
            "wv": wv_l,
            "wn8": wn8,
        })
    return in_maps, w_last


def kernel(k, q, Wk, Wv):
    from concourse.bass_utils import run_bass_kernel_spmd

    in_maps, w_last = make_in_maps(k, q, Wk, Wv)
    nc = _get_program(w_last)
    res = run_bass_kernel_spmd(nc, in_maps, core_ids=list(range(N_CORES)))
    return np.concatenate([res.results[c]["out"] for c in range(N_CORES)],
                          axis=0)


# revision 33
# speedup vs baseline: 1.0638x; 1.0638x over previous
# Bass/Tile kernel for nn_LongTermAttention (continuous long-term attention
# with rectangular basis functions) on 8 Trainium2 NeuronCores.
#
# Mathematical rewrite (verified exact vs the reference):
#   * G = F^T (F F^T + ridge I)^{-1} for the rectangular basis on the padded
#     uniform grid collapses to G[l, n] = (1/4.5) * [l // 4 == n], so
#     Bc[b,n,e] = (1/4.5) * sum_{j<4} k[b,e,4n+j]  (4-wide sum pooling).
#   * psi on the integration grid is a one-hot selector, so the P=1000-point
#     continuous softmax reduces to basis space with quadrature mass Wn per
#     basis:  p_n = exp(s_n) Wn / Z,  Z = sum_n exp(s_n) Wn + w_last.
#     Wn is folded into the VALUES tiles (and the Z "ones" column), so the
#     device computes a bias-free exp.
#   * The max-subtraction in the reference cancels exactly.
#
# Performance structure (v3):
#   * k stays in original [b, e, l] layout; pooling via single-pass
#     vector.pool_avg (innermost-dim reduce, x4 folded into weight scales),
#     with gpsimd covering half-tiles via a 2-op add tree.
#   * All SBUF tiles are persistent (unique tags) so no DMA trigger ever
#     blocks on tile recycling.
#   * Scores for head-pair m are emitted right after keysT[m] drains, so the
#     scalar-engine exp stream (the critical tail) starts as early as
#     possible; exp is the ONLY scalar work.
#   * PE warm-up matmuls (zeros) ramp the tensor-engine p-state before real
#     data lands; emission order keeps the PE busy throughout.
#   * Normalize: vector does psum->sbuf cast + Z reciprocal; gpsimd does the
#     per-head scaling from SBUF.
#
# Sharding: data-parallel over batch, 2 batches per core; weights replicated.

import numpy as np

B_FULL = 16
N_CORES = 8
B_PER = B_FULL // N_CORES  # 2
E = 512          # embed dim
L = 2048         # memory length
T = 256          # query length
N = 512          # basis count
H = 8            # heads
D = 64           # head dim
P_GRID = 1000    # integration points
RIDGE_C = 4.5    # F F^T diag (4.0) + ridge (0.5)

N_WARM1 = 12     # junk matmuls before first real matmul
N_WARM2 = 3      # junk matmuls inside the first keysT chain gap
N_WARM3 = 6      # junk matmuls while waiting for late b0 k tiles

_CACHE = {}


def _host_constants(Wk, Wv):
    """Pre-scale and lay out weights; build quadrature-mass vectors.
    Scales fold: pooling 1/4.5 and query scale 1/8 into Wk; 1/4.5 into Wv
    (device pooling is a plain 4-wide SUM)."""
    import ml_dtypes
    bf16 = ml_dtypes.bfloat16
    wk = (Wk.astype(np.float64) / (RIDGE_C * 8.0)).astype(np.float32)
    wv = (Wv.astype(np.float64) / RIDGE_C).astype(np.float32)
    # layout [p, kk, e'] with e = kk*128 + p
    wk_l = np.ascontiguousarray(
        wk.reshape(4, 128, 512).transpose(1, 0, 2)).astype(bf16)
    wv_l = np.ascontiguousarray(
        wv.reshape(4, 128, 512).transpose(1, 0, 2)).astype(bf16)
    # quadrature mass per basis (trapezoid weights summed per bin, p<999)
    p = np.arange(P_GRID)
    nmap = (512 * p) // 999
    w = np.full(P_GRID, 1.0 / 999.0)
    w[0] = w[-1] = 1.0 / 1998.0
    Wn = np.zeros(N)
    for i in range(P_GRID - 1):
        Wn[nmap[i]] += w[i]
    # wn8[p, ms, h] = Wn[ms*128 + p] replicated over 8 heads
    wn8 = np.ascontiguousarray(
        np.repeat(Wn.reshape(4, 128).T[:, :, None], H, axis=2)
    ).astype(np.float32)
    w_last = float(w[-1])
    return wk_l, wv_l, wn8, w_last


def _build_program(w_last):
    import concourse.bass as bass
    import concourse.mybir as mybir
    import concourse.tile as tile
    from concourse import bacc

    nc = bacc.Bacc(
        "TRN2",
        target_bir_lowering=False,
        debug=False,
        enable_asserts=False,
        num_devices=N_CORES,
    )

    f32 = mybir.dt.float32
    bf16 = mybir.dt.bfloat16

    k_d = nc.dram_tensor("k", [B_PER, 4, 128, L], bf16,
                         kind="ExternalInput").ap()
    q_d = nc.dram_tensor("q", [B_PER, 128, 4, T], bf16,
                         kind="ExternalInput").ap()
    wk_d = nc.dram_tensor("wk", [128, 4, E], bf16, kind="ExternalInput").ap()
    wv_d = nc.dram_tensor("wv", [128, 4, E], bf16, kind="ExternalInput").ap()
    wn8_d = nc.dram_tensor("wn8", [128, 4, H], f32, kind="ExternalInput").ap()
    out_d = nc.dram_tensor("out", [B_PER, T, E], f32,
                           kind="ExternalOutput").ap()

    from contextlib import ExitStack
    with tile.TileContext(nc) as tc, ExitStack() as ctx:
        _kernel_body(ctx, tc, nc, mybir, k_d, q_d, wk_d, wv_d, wn8_d, out_d,
                     w_last)

    nc.compile()
    return nc


def _kernel_body(ctx, tc, nc, mybir, k_d, q_d, wk_d, wv_d, wn8_d, out_d,
                 w_last):
    f32 = mybir.dt.float32
    bf16 = mybir.dt.bfloat16
    Exp = mybir.ActivationFunctionType.Exp
    AVG = mybir.PoolFunctionType.avg

    def pool(name, bufs, space="SBUF"):
        return ctx.enter_context(tc.tile_pool(name=name, bufs=bufs,
                                              space=space))

    # every SBUF tile is persistent (unique tag) so nothing ever blocks on
    # tile recycling
    sb = pool("sb", 1)
    t1p = pool("t1p", 4)
    ps_proj = pool("ps_proj", 2, "PSUM")  # [128, 512] f32 (1 bank each)
    ps_sc = pool("ps_sc", 2, "PSUM")      # [128, 1024] f32 (2 banks each)
    ps_ctx = pool("ps_ctx", 2, "PSUM")    # [128, 260] f32

    def sbt(shape, dtype, tag):
        return sb.tile(shape, dtype, tag=tag, name=tag)

    # ---------------- constants / DMA priority queues ----------------
    # act ring:  wn8, wk_h0, k(b0,1)h0, wk_h1, k(b0,1)h1, wv, k(b1,3), b0-out
    # sync ring: k(b0,0)h0, k(b0,2)h0, k(b0,0)h1, k(b0,2)h1, k(b1,0),
    #            k(b1,2), q1, b1-out
    # gpsimd SW ring: q0, k(b0,3), k(b1,1)
    wn8_sb = sbt([128, 4 * H], f32, "wn8")
    wk_sb = sbt([128, 4 * E], bf16, "wk")
    nc.scalar.dma_start(wk_sb[:].rearrange("p (kk e) -> p kk e", kk=4),
                        wk_d[:])
    wv_sb = sbt([128, 4 * E], bf16, "wv")
    qT = [sbt([128, 4 * T], bf16, f"qT{b}") for b in range(B_PER)]
    junk_sb = sbt([128, 512], bf16, "junk")
    nc.vector.memset(junk_sb[:], 0.0)

    # k tiles: kth[(b, et, h)] is the SBUF view of half h (n-range
    # h*256:(h+1)*256, i.e. l-range h*1024:(h+1)*1024)
    kth = {}

    def dma_k_full(b, et, eng):
        kt = sbt([128, L], bf16, f"k{b}_{et}")
        eng.dma_start(kt[:], k_d[b, et])
        kth[(b, et, 0)] = kt[:, 0:1024]
        kth[(b, et, 1)] = kt[:, 1024:2048]

    # sync handles two tiles fast; act takes wk+et1; the slow SW ring
    # gets et3 first (lands ~15us) plus b1's most-slack tile
    dma_k_full(0, 3, nc.gpsimd)
    dma_k_full(1, 1, nc.gpsimd)
    # sync ring
    dma_k_full(0, 0, nc.sync)
    dma_k_full(0, 2, nc.sync)
    nc.sync.dma_start(qT[0][:].rearrange("p (e t) -> p e t", e=4), q_d[0])
    nc.sync.dma_start(wv_sb[:].rearrange("p (kk e) -> p kk e", kk=4),
                      wv_d[:])
    dma_k_full(1, 0, nc.sync)
    dma_k_full(1, 2, nc.sync)
    nc.sync.dma_start(qT[1][:].rearrange("p (e t) -> p e t", e=4), q_d[1])
    # act ring (wk above; wn8 is only needed by the Z columns ~20us in)
    dma_k_full(0, 1, nc.scalar)
    nc.scalar.dma_start(wn8_sb[:].rearrange("p (m h) -> p m h", m=4),
                        wn8_d[:])
    dma_k_full(1, 3, nc.scalar)

    # ---------------- values tiles + persistent Z columns ----------------
    values = {(b, ms): sbt([128, H * 65], bf16, f"val{b}_{ms}")
              for b in range(B_PER) for ms in range(4)}
    wn8v = wn8_sb[:].rearrange("p (m h) -> p m h", m=4)

    def z_cols():
        for b in range(B_PER):
            for ms in range(4):
                vv = values[(b, ms)][:].rearrange("p (h c) -> p h c", c=65)
                nc.gpsimd.tensor_copy(vv[:, :, 64], wn8v[:, ms, :])

    # ---------------- pooling (4-wide sum; per n-half) ----------------
    # pl[(b, et, c)] [128, 256]; vector handles all c=0 halves via
    # tensor_reduce, gpsimd all c=1 halves via a 2-op add tree.
    pl = {(b, et, c): sbt([128, 256], bf16, f"pl{b}_{et}_{c}")
          for b in range(B_PER) for et in range(4) for c in range(2)}
    AXX = mybir.AxisListType.X
    ADD = mybir.AluOpType.add

    def pool_v(b, et, c):
        kv = kth[(b, et, c)].rearrange("p (n j) -> p n j", j=4)
        with nc.allow_low_precision("4-wide pooling sum in bf16"):
            nc.vector.tensor_reduce(pl[(b, et, c)][:], kv, axis=AXX, op=ADD)

    def pool_g(b, et, c):
        kv = kth[(b, et, c)].rearrange("p (n j) -> p n j", j=4)
        t1 = t1p.tile([128, 512], bf16, tag="t1", name=f"t1_{b}_{et}_{c}")
        t1v = t1[:].rearrange("p (n j) -> p n j", j=2)
        nc.gpsimd.tensor_add(t1v[:, :, :], kv[:, :, 0:2], kv[:, :, 2:4])
        nc.gpsimd.tensor_add(pl[(b, et, c)][:], t1v[:, :, 0], t1v[:, :, 1])

    # b0 pooling: c0 halves split across vector (et0,et2) and gpsimd
    # (et1,et3) so the critical first half completes fastest; c1 halves of
    # et1/et3 also on gpsimd (emitted now), et0/et2-c1 on vector later so
    # the keysT-c0 drains can jump the vector queue.
    pool_v(0, 0, 0)
    pool_v(0, 2, 0)
    pool_v(0, 3, 0)
    pool_g(0, 1, 0)
    pool_g(0, 1, 1)
    pool_g(0, 3, 1)
    z_cols()

    # ---------------- PE warm-up ----------------
    def junk_block(n):
        ps = ps_proj.tile([128, 512], f32, tag="ps_proj", name="junk")
        for _ in range(n):
            nc.tensor.matmul(ps[:], junk_sb[:, 0:128], junk_sb[:],
                             start=True, stop=True, skip_group_check=True)

    # ---------------- projections (per half) ----------------
    keysT = {}

    def kT_chain(b, m, c, order, warm=0):
        ps = ps_proj.tile([128, 512], f32, tag="ps_proj",
                          name=f"kT{b}_{m}_{c}")
        for i, kk in enumerate(order):
            if warm and i == 2:
                junk_block(warm)
            nc.tensor.matmul(
                ps[:, 0:256],
                wk_sb[:, kk * E + m * 128: kk * E + (m + 1) * 128],
                pl[(b, kk, c)][:],
                start=(i == 0), stop=(i == 3),
            )
        kt_sb = sbt([128, 256], bf16, f"kT{b}_{m}_{c}")
        nc.vector.tensor_copy(kt_sb[:], ps[:, 0:256])
        keysT[(b, m, c)] = kt_sb

    def val_chain(b, ms, order):
        ps = ps_proj.tile([128, 512], f32, tag="ps_proj", name=f"v{b}_{ms}")
        for i, kk in enumerate(order):
            nc.tensor.matmul(
                ps[:],
                pl[(b, kk, ms // 2)][:, (ms % 2) * 128:(ms % 2) * 128 + 128],
                wv_sb[:, kk * E:(kk + 1) * E],
                start=(i == 0), stop=(i == 3),
            )
        vv = values[(b, ms)][:].rearrange("p (h c) -> p h c", c=65)
        nc.vector.tensor_scalar_mul(
            vv[:, :, 0:64],
            ps[:].rearrange("p (h d) -> p h d", d=64),
            wn8_sb[:, ms * H: ms * H + 1],
        )

    # ---------------- scores + exp ----------------
    u_tiles = {}

    def score_block(b, hp, ab):
        ps = ps_sc.tile([128, 1024], f32, tag="ps_sc", name=f"sc{b}_{hp}_{ab}")
        for nbp in range(2):
            for h01 in range(2):
                nc.tensor.matmul(
                    ps[:, h01 * 512 + nbp * 256: h01 * 512 + nbp * 256 + 256],
                    keysT[(b, hp, ab)][h01 * 64:(h01 + 1) * 64,
                                       nbp * 128:(nbp + 1) * 128],
                    qT[b][h01 * 64:(h01 + 1) * 64,
                          hp * 256:(hp + 1) * 256],
                    start=True, stop=True,
                    skip_group_check=True,
                )
        u = sbt([128, 1024], bf16, f"u{b}_{hp}_{ab}")
        nc.scalar.activation(u[:], ps[:], Exp)
        u_tiles[(b, hp, ab)] = u

    # ---------------- ctx + normalize ----------------
    out_sbs = {(b, mb): sbt([128, E], f32, f"out{b}_{mb}")
               for b in range(B_PER) for mb in range(2)}
    Copy = mybir.ActivationFunctionType.Copy

    def ctx_block(b, hp):
        pc = ps_ctx.tile([128, 260], f32, tag="ps_ctx", name=f"ctx{b}_{hp}")
        for h01 in range(2):
            h = hp * 2 + h01
            for mb in range(2):
                c = h01 * 2 + mb
                for nb in range(4):
                    nc.tensor.matmul(
                        pc[:, c * 65:(c + 1) * 65],
                        u_tiles[(b, hp, nb // 2)][
                            :, h01 * 512 + (nb % 2) * 256 + mb * 128:
                            h01 * 512 + (nb % 2) * 256 + (mb + 1) * 128],
                        values[(b, nb)][:, h * 65: h * 65 + 65],
                        start=(nb == 0), stop=(nb == 3),
                        skip_group_check=True,
                    )
        pcv = pc[:].rearrange("p (c x) -> p c x", x=65)
        rz = sbt([128, 4], f32, f"rz{b}_{hp}")
        nc.vector.tensor_scalar_add(rz[:], pcv[:, :, 64], w_last)
        rzi = sbt([128, 4], f32, f"rzi{b}_{hp}")
        nc.vector.reciprocal(rzi[:], rz[:])
        for h01 in range(2):
            h = hp * 2 + h01
            for mb in range(2):
                c = h01 * 2 + mb
                dst = out_sbs[(b, mb)][:, h * 64:(h + 1) * 64]
                nc.vector.tensor_scalar_mul(dst, pcv[:, c, 0:64],
                                            rzi[:, c:c + 1])

    # ---------------- emission ----------------
    C0_B0 = (0, 2, 3, 1)    # b0 c0-half pooled-arrival order
    C1_B0 = (0, 2, 1, 3)    # b0 c1-half pooled-arrival order
    C0_B1 = (1, 3, 0, 2)
    C1_B1 = (1, 3, 0, 2)

    junk_block(N_WARM1)
    # b0 keysT c0 chains; m0/m1 drains jump the vector queue before the
    # vector picks up the et0/et2 c1-half reductions
    kT_chain(0, 0, 0, C0_B0, warm=N_WARM2)
    kT_chain(0, 1, 0, C0_B0)
    pool_v(0, 0, 1)
    pool_v(0, 2, 1)
    kT_chain(0, 2, 0, C0_B0)
    kT_chain(0, 3, 0, C0_B0)
    # all four A score blocks are ready now; emit them ahead of the
    # (possibly stalling) c1 keysT chains so the exp stream starts early
    score_block(0, 0, 0)
    score_block(0, 1, 0)
    kT_chain(0, 0, 1, C1_B0)
    score_block(0, 2, 0)
    kT_chain(0, 1, 1, C1_B0)
    score_block(0, 3, 0)
    kT_chain(0, 2, 1, C1_B0)
    kT_chain(0, 3, 1, C1_B0)
    # b1 pooling: gpsimd trees for et1/et3 c0 and all c1; vector c0 reds
    # for et0/et2 come after the b0 value drains (emitted below)
    pool_g(1, 1, 0)
    pool_g(1, 3, 0)
    pool_g(1, 1, 1)
    pool_g(1, 3, 1)
    pool_g(1, 0, 1)
    pool_g(1, 2, 1)
    pool_v(1, 0, 0)
    pool_v(1, 2, 0)
    # b1 keysT-c0 chains fill between the b0 B-score blocks; the b0 value
    # chains (needed only by ctx) fill between the b1 A-score blocks
    score_block(0, 0, 1)
    kT_chain(1, 0, 0, C0_B1)
    score_block(0, 1, 1)
    kT_chain(1, 1, 0, C0_B1)
    score_block(0, 2, 1)
    kT_chain(1, 2, 0, C0_B1)
    score_block(0, 3, 1)
    kT_chain(1, 3, 0, C0_B1)
    score_block(1, 0, 0)
    val_chain(0, 0, C0_B0)
    score_block(1, 1, 0)
    val_chain(0, 1, C0_B0)
    score_block(1, 2, 0)
    val_chain(0, 2, C1_B0)
    score_block(1, 3, 0)
    val_chain(0, 3, C1_B0)
    kT_chain(1, 0, 1, C1_B1)
    kT_chain(1, 1, 1, C1_B1)
    ctx_block(0, 0)
    kT_chain(1, 2, 1, C1_B1)
    kT_chain(1, 3, 1, C1_B1)
    ctx_block(0, 1)
    score_block(1, 0, 1)
    val_chain(1, 0, C0_B1)
    score_block(1, 1, 1)
    val_chain(1, 1, C0_B1)
    for mb in range(2):
        nc.scalar.dma_start(out_d[0, mb * 128:(mb + 1) * 128, 0:256],
                            out_sbs[(0, mb)][:, 0:256])
    ctx_block(0, 2)
    score_block(1, 2, 1)
    val_chain(1, 2, C1_B1)
    score_block(1, 3, 1)
    val_chain(1, 3, C1_B1)
    ctx_block(0, 3)
    for mb in range(2):
        nc.scalar.dma_start(out_d[0, mb * 128:(mb + 1) * 128, 256:512],
                            out_sbs[(0, mb)][:, 256:512])

    ctx_block(1, 0)
    ctx_block(1, 1)
    for mb in range(2):
        nc.sync.dma_start(out_d[1, mb * 128:(mb + 1) * 128, 0:256],
                          out_sbs[(1, mb)][:, 0:256])
    ctx_block(1, 2)
    ctx_block(1, 3)
    for mb in range(2):
        nc.sync.dma_start(out_d[1, mb * 128:(mb + 1) * 128, 256:512],
                          out_sbs[(1, mb)][:, 256:512])


def _get_program(w_last):
    if "nc" not in _CACHE:
        _CACHE["nc"] = _build_program(w_last)
    return _CACHE["nc"]


def make_in_maps(k, q, Wk, Wv):
    import ml_dtypes
    bf16 = ml_dtypes.bfloat16
    wk_l, wv_l, wn8, w_last = _host_constants(Wk, Wv)
    k16 = np.asarray(k).astype(bf16)
    q16 = np.asarray(q).astype(bf16)
    in_maps = []
    for c in range(N_CORES):
        ks = np.ascontiguousarray(
            k16[c * B_PER:(c + 1) * B_PER].reshape(B_PER, 4, 128, L))
        qs = q16[c * B_PER:(c + 1) * B_PER]          # [2, 256, 512]
        # -> [b, p, eb, t]
        qp = np.ascontiguousarray(
            qs.transpose(0, 2, 1).reshape(B_PER, 4, 128, T)
            .transpose(0, 2, 1, 3))
        in_maps.append({
            "k": ks,
            "q": qp,
            "wk": wk_l,
            "wv": wv_l,
            "wn8": wn8,
        })
    return in_maps, w_last


def kernel(k, q, Wk, Wv):
    from concourse.bass_utils import run_bass_kernel_spmd

    in_maps, w_last = make_in_maps(k, q, Wk, Wv)
    nc = _get_program(w_last)
    res = run_bass_kernel_spmd(nc, in_maps, core_ids=list(range(N_CORES)))
    return np.concatenate([res.results[c]["out"] for c in range(N_CORES)],
                          axis=0)
